# revision 50
# baseline (speedup 1.0000x reference)
"""Trainium2 Bass kernel for the LogRatio loss (nn_LogRatio_14104672600201).

Math: the reference loss factorizes. Every masked reduction over
logsim[j, l] = log((X @ X.T)[j, l] + eps) has a mask that depends on j only
through targets[j] in [0, 64), so each row-reduction becomes a small GEMM
against matrices built on host from labels. After folding all shift / diag /
count terms into host-precomputed coefficients, the per-row loss is

  loss_j = -2*s1p*s1w0 + k2_j*s1w0 + selU_j + selV_j + k0_j

where (with y = logsim - s, s = 3.5 shift):
  X1[j, 0:192] = sum_l y[l, j] * [P | W0 | U*su][l, :]  (q-as-N accumulation)
  s1p/s1w0/selU = X1[j, g + t_j] for g = 0/64/128      (host gather)
  X2[c, j]     = sum_l y[l, j]^2 * (V*sv)[l, c]        (q-major accumulation)
  sum_j selV_j = sum_{c,j} X2[c, j] * phT[c, j] / sv   (host dot on raw X2)
  K0 = sum_j k0_j is a pure-host constant.
The device only produces the raw X1/X2 slabs; the t_j gathers, the bilinear
product and the final reduction run on host in f64 — O(N*C) work against
the device's O(N^2*C).

U[l, c] = -0.2*h1[c]*P[l, c] + 0.2*(cnt[c]-1)*W1[c, l] and
V[l, c] = h0[c]*P[l, c] + (cnt[c]-1)*W0[c, l] absorb the linear terms whose
per-row coefficients depend on j only through t_j; su/sv scale them into
fp8e4 range (TRN max-normal 240), compensated in kt / on host.

Engine plan per l-tile pair (16 pairs):
  - 2 sim matmuls (bf16, N=512) into one [128, 1024] 2-bank PSUM tile
  - 1 activation Ln over the full [128, 1024] tile -> ls pair (fp8)
  - 4 DoubleRow matmuls (fp8, K=128x2) accumulate X1 for the 4 j-chunks
  - 1 DVE square -> ls2 pair (fp8), 1 DoubleRow matmul accumulates X2
At the end the two slabs are copied PSUM->SBUF on the scalar and vector
engines in parallel and DMA'd out on separate queues.
A short burst of dummy matmuls during the DMA prologue keeps the PE's HAM
activity window hot so the loop starts at the full 2.4GHz clock.
The four X1 accumulators share two PSUM banks: the jc0/jc2 matmul starts
each bank's accumulation group (pending-zeroing the whole bank) and the
jc1/jc3 matmuls never start, relying on PE program order for first-write
zeroing — CoreSim's group checker verifies this ordering at build time.

Sharding: data-parallel over j across 8 cores (512 rows each). Every core
holds the full xT bf16; the scalar loss is reduced on host (the
"all-reduce" of a scalar).
"""

import numpy as np

N, D, KK, C = 4096, 128, 4, 64
NCORES = 8
JSH = N // NCORES          # 512 j rows per core
JCH = 4                    # j-chunks of 128 per core
LT = N // 128              # 32 l-tiles
LTP = LT // 2              # 16 l-tile pairs
QW = 192                   # x1 width: P(64) | W0(64) | U(64)
ACCW = 256                 # per-jc accumulator stride (half a PSUM bank)
NSEL = 3                   # selections per row from X1
AUXW = 68                  # aux cols: 64 one-hot + 3 ktab + 1 pad
EPS = 1e-6
OMEGA = 0.1
KSC = float(np.float32(np.exp(-3.5)))        # Ln input scale (exactly f32)
SHIFT = float(-np.log(np.float64(KSC)))      # effective shift s = -ln(KSC)

_CACHE = {}


def _build_nc():
    import concourse.bass as bass
    import concourse.bacc as bacc
    import concourse.mybir as mybir
    import concourse.tile as tile
    from contextlib import ExitStack

    f32 = mybir.dt.float32
    bf16 = mybir.dt.bfloat16
    fp8 = mybir.dt.float8e4
    DR = mybir.MatmulPerfMode.DoubleRow
    Ln = mybir.ActivationFunctionType.Ln
    mult = mybir.AluOpType.mult
    add = mybir.AluOpType.add
    AxX = mybir.AxisListType.X

    nc = bacc.Bacc("TRN2", target_bir_lowering=False, debug=False)
    xt = nc.dram_tensor("xt", [D, N], bf16, kind="ExternalInput")
    # q pairs: row p holds [q(l0) | q(l1)] per l-tile pair, fp8
    qp = nc.dram_tensor("qp", [128, LT * QW], fp8, kind="ExternalInput")
    # V pairs: row p holds [v(l0) | v(l1)] per l-tile pair, fp8
    vp = nc.dram_tensor("vp", [128, LT * C], fp8, kind="ExternalInput")
    # raw X1 / X2 slabs go to host; gathers + bilinear term are O(N) there
    x1out = nc.dram_tensor("x1out", [128, JCH * QW], bf16, kind="ExternalOutput")
    xvout = nc.dram_tensor("xvout", [C, JSH], bf16, kind="ExternalOutput")

    with tile.TileContext(nc) as tc, ExitStack() as ctx:
        cpool = ctx.enter_context(tc.tile_pool(name="const", bufs=1))
        work = ctx.enter_context(tc.tile_pool(name="work", bufs=5))
        small = ctx.enter_context(tc.tile_pool(name="small", bufs=2))
        psim = ctx.enter_context(tc.tile_pool(name="psim", bufs=2, space="PSUM"))
        pacc = ctx.enter_context(tc.tile_pool(name="pacc", bufs=1, space="PSUM"))

        # ---- constants in SBUF; first-needed bytes land first ----
        xt_sb = cpool.tile([D, N], bf16, tag="xt")
        qp_sb = cpool.tile([128, LT * QW], fp8, tag="qp")
        vp_sb = cpool.tile([128, LT * C], fp8, tag="vp")
        nc.sync.dma_start(xt_sb[:, 0:512], xt[:, 0:512])
        nc.sync.dma_start(qp_sb[:, 0 : 4 * QW], qp[:, 0 : 4 * QW])
        nc.sync.dma_start(vp_sb[:, 0 : 4 * C], vp[:, 0 : 4 * C])
        nc.sync.dma_start(xt_sb[:, 512:1024], xt[:, 512:1024])
        nc.sync.dma_start(xt_sb[:, 1024:2048], xt[:, 1024:2048])
        nc.sync.dma_start(qp_sb[:, 4 * QW : 12 * QW], qp[:, 4 * QW : 12 * QW])
        nc.sync.dma_start(vp_sb[:, 4 * C :], vp[:, 4 * C :])
        nc.sync.dma_start(xt_sb[:, 2048:3072], xt[:, 2048:3072])
        nc.sync.dma_start(qp_sb[:, 12 * QW : 22 * QW], qp[:, 12 * QW : 22 * QW])
        nc.sync.dma_start(xt_sb[:, 3072:4096], xt[:, 3072:4096])
        nc.sync.dma_start(qp_sb[:, 22 * QW :], qp[:, 22 * QW :])
        qp3 = qp_sb[:].rearrange("p (t s c) -> p t s c", t=LTP, s=2)
        vp3 = vp_sb[:].rearrange("p (t s c) -> p t s c", t=LTP, s=2)

        # ---- accumulators: 4 X1 chunks packed two-per-bank (jc0/jc2 start
        # the bank's group, jc1/jc3 ride along), X2 in its own bank ----
        acc = pacc.tile([128, JCH * ACCW], f32, tag="acc")
        xv = pacc.tile([C, JSH], f32, tag="xv")

        # ---- PE warm-up in the spare PSUM bank: keep the HAM activity
        # window hot through the DMA prologue so the loop starts at full
        # clock instead of ramping through its first ~5us ----
        warm_sb = cpool.tile([128, 256], bf16, tag="warm")
        nc.gpsimd.memset(warm_sb[:], 0.0)
        warmp = pacc.tile([128, 256], f32, tag="warmp")
        for _ in range(4):
            nc.tensor.matmul(
                warmp[:], warm_sb[:, 0:128], warm_sb[:], start=True, stop=True
            )

        for ltp in range(LTP):
            st = ltp == 0
            sp = ltp == LTP - 1
            simp = psim.tile([128, 2 * JSH], f32, tag="simp", name=f"simp{ltp}")
            for s in range(2):
                lt = 2 * ltp + s
                nc.tensor.matmul(
                    simp[:, s * JSH : (s + 1) * JSH],
                    xt_sb[:, bass.ts(lt, 128)],
                    xt_sb[:, 0:JSH],
                    start=True,
                    stop=True,
                )
            lsp = work.tile([128, 2 * JSH], fp8, tag="lsp", name=f"lsp{ltp}")
            nc.scalar.activation(lsp[:], simp[:], Ln, scale=KSC)
            lsp3 = lsp[:].rearrange("p (s j) -> p s j", s=2)
            for jc in range(JCH):
                nc.tensor.matmul(
                    acc[:, jc * ACCW : jc * ACCW + QW],
                    lsp3[:, :, bass.ts(jc, 128)],
                    qp3[:, ltp],
                    start=st and jc % 2 == 0,
                    stop=sp and jc % 2 == 1,
                    perf_mode=DR,
                )
            ls2p = work.tile([128, 2 * JSH], fp8, tag="ls2p", name=f"ls2p{ltp}")
            nc.vector.tensor_mul(ls2p[:], lsp[:], lsp[:])
            ls2p3 = ls2p[:].rearrange("p (s j) -> p s j", s=2)
            nc.tensor.matmul(
                xv[:], vp3[:, ltp], ls2p3, start=st, stop=sp, perf_mode=DR
            )

        # ---- raw slabs out: copies run on ACT and DVE in parallel ----
        ob1 = small.tile([128, JCH * QW], bf16, tag="ob1")
        nc.scalar.copy(
            ob1[:].rearrange("p (j c) -> p j c", j=JCH),
            acc[:].rearrange("p (j c) -> p j c", j=JCH)[:, :, 0:QW],
        )
        nc.sync.dma_start(x1out[:], ob1[:])
        ob2 = small.tile([C, JSH], bf16, tag="ob2")
        nc.vector.tensor_copy(ob2[:], xv[:])
        nc.sync.dma_start(xvout[:], ob2[:])
    nc.compile()
    return nc


def _host_prep(inputs, labels):
    import ml_dtypes

    bf16 = ml_dtypes.bfloat16
    fp8 = ml_dtypes.float8_e4m3
    x = np.ascontiguousarray(np.asarray(inputs, dtype=np.float32))
    lab = np.asarray(labels)
    t = lab[:, 0]

    m = np.arange(KK)
    lp = np.log(np.float64(OMEGA + EPS)) - np.log(
        np.float64(OMEGA) ** (KK - m + 1) + np.float64(EPS)
    )

    gr = np.arange(C)
    eq = lab[None, :, :] == gr[:, None, None]          # [C, N, KK]
    nm = np.stack(
        [
            ~eq[:, :, 3],
            eq[:, :, 3] & ~eq[:, :, 2],
            eq[:, :, 2] & ~eq[:, :, 1],
            eq[:, :, 1] & ~eq[:, :, 0],
        ]
    ).astype(np.float64)                                # [KK, C, N]
    w0 = nm.sum(0)                                      # [C, N]
    w1 = np.einsum("m,mcl->cl", lp, nm)
    w2 = np.einsum("m,mcl->cl", lp * lp, nm)
    ph = (t[:, None] == gr[None, :]).astype(np.float64)  # [N, C] one-hot t_l

    h0 = w0.sum(1)
    h1 = w1.sum(1)
    h2 = w2.sum(1)
    cnt = ph.sum(0)
    s = np.float64(SHIFT)

    # diag in logsim-domain from the bf16-rounded x the device will see
    xb = x.astype(bf16).astype(np.float64)
    diag = np.log((xb * xb).sum(1) + EPS)                # [N]

    a0 = h0[t]
    cc = cnt[t] - 1.0
    u1 = s * cnt[t] - diag
    u2 = s * s * cnt[t] - diag * diag
    u3 = s * h0[t] + 0.1 * h1[t]
    u4 = s * s * h0[t] + 0.2 * s * h1[t] + 0.01 * h2[t]
    k0 = (a0 * u2 - 2.0 * u1 * u3 + cc * u4).sum()       # host constant
    k2 = 2.0 * diag - 2.0 * s

    # fold t_j-only linear terms into U (for X1) and V (for X2), scaled
    # into fp8e4 range (TRN max-normal 240)
    U = -0.2 * h1[None, :] * ph + 0.2 * (cnt[None, :] - 1.0) * w1.T
    V = h0[None, :] * ph + (cnt[None, :] - 1.0) * w0.T
    su = 2.0 ** np.floor(np.log2(200.0 / np.abs(U).max()))
    sv = 2.0 ** np.floor(np.log2(200.0 / np.abs(V).max()))

    qm = np.concatenate([ph, w0.T, U * su], axis=1).astype(fp8)  # [N, 192]
    vm = (V * sv).astype(fp8)                                    # [N, 64]

    xt = np.ascontiguousarray(x.T.astype(bf16))          # [D, N] bf16
    in_maps = []
    for cid in range(NCORES):
        sl = slice(cid * JSH, (cid + 1) * JSH)
        # rotate the l axis so this core's own j-shard sits at columns
        # 0:JSH — the kernel always matmuls against xt[:, 0:512]; the l
        # reduction (over all 4096) is rotation-invariant as long as q/v
        # rows rotate identically.
        xtc = np.ascontiguousarray(np.roll(xt, -cid * JSH, axis=1))
        # pair images [128, ltp, s, width]
        qpc = np.ascontiguousarray(
            np.roll(qm, -cid * JSH, axis=0)
            .reshape(LTP, 2, 128, QW).transpose(2, 0, 1, 3)
            .reshape(128, LT * QW)
        )
        vpc = np.ascontiguousarray(
            np.roll(vm, -cid * JSH, axis=0)
            .reshape(LTP, 2, 128, C).transpose(2, 0, 1, 3)
            .reshape(128, LT * C)
        )
        in_maps.append({"xt": xtc, "qp": qpc, "vp": vpc})
    host = {"k0": float(k0), "k2": k2, "t": t, "ph": ph,
            "su": float(su), "sv": float(sv)}
    return in_maps, host


def _run(inputs, labels, trace=False, tmpdir=None):
    from concourse.bass_utils import run_bass_kernel_spmd

    if "nc" not in _CACHE:
        _CACHE["nc"] = _build_nc()
    in_maps, host = _host_prep(inputs, labels)
    res = run_bass_kernel_spmd(
        _CACHE["nc"], in_maps, core_ids=list(range(NCORES)),
        trace=trace, tmpdir=tmpdir,
    )
    k2, t, ph = host["k2"], host["t"], host["ph"]
    su, sv = host["su"], host["sv"]
    loss = np.float64(host["k0"])
    jj = np.arange(JSH)
    for cid, r in enumerate(res.results):
        sl = slice(cid * JSH, (cid + 1) * JSH)
        # j-major slab: row p, block jc holds X1[j = jc*128 + p, :]
        a = (r["x1out"].astype(np.float64)
             .reshape(128, JCH, QW).transpose(1, 0, 2).reshape(JSH, QW))
        tj = t[sl]
        s1p = a[jj, tj]
        s1w0 = a[jj, 64 + tj]
        selu = a[jj, 128 + tj] / su
        loss += (-2.0 * s1p * s1w0 + k2[sl] * s1w0 + selu).sum()
        loss += (r["xvout"].astype(np.float64) * ph[sl].T).sum() / sv
    return np.array(loss, dtype=np.float32), res


def kernel(inputs, labels):
    out, _ = _run(inputs, labels, trace=False)
    return out


# revision 51
# speedup vs baseline: 1.0136x; 1.0136x over previous
"""Trainium2 Bass kernel for the LogRatio loss (nn_LogRatio_14104672600201).

Math: the reference loss factorizes. Every masked reduction over
logsim[j, l] = log((X @ X.T)[j, l] + eps) has a mask that depends on j only
through targets[j] in [0, 64), so each row-reduction becomes a small GEMM
against matrices built on host from labels. After folding all shift / diag /
count terms into host-precomputed coefficients, the per-row loss is

  loss_j = -2*s1p*s1w0 + k2_j*s1w0 + selU_j + selV_j + k0_j

where (with y = logsim - s, s = 3.5 shift):
  X1[j, 0:192] = sum_l y[l, j] * [P | W0 | U*su][l, :]  (q-as-N accumulation)
  s1p/s1w0/selU = X1[j, g + t_j] for g = 0/64/128      (host gather)
  X2[c, j]     = sum_l y[l, j]^2 * (V*sv)[l, c]        (q-major accumulation)
  sum_j selV_j = sum_{c,j} X2[c, j] * phT[c, j] / sv   (host dot on raw X2)
  K0 = sum_j k0_j is a pure-host constant.
The device only produces the raw X1/X2 slabs; the t_j gathers, the bilinear
product and the final reduction run on host in f64 — O(N*C) work against
the device's O(N^2*C).

U[l, c] = -0.2*h1[c]*P[l, c] + 0.2*(cnt[c]-1)*W1[c, l] and
V[l, c] = h0[c]*P[l, c] + (cnt[c]-1)*W0[c, l] absorb the linear terms whose
per-row coefficients depend on j only through t_j; su/sv scale them into
fp8e4 range (TRN max-normal 240), compensated in kt / on host.

Engine plan per l-tile pair (16 pairs):
  - 2 sim matmuls (bf16, N=512) into one [128, 1024] 2-bank PSUM tile
  - 1 activation Ln over the full [128, 1024] tile -> ls pair (fp8)
  - 4 DoubleRow matmuls (fp8, K=128x2) accumulate X1 for the 4 j-chunks
  - 1 DVE square -> ls2 pair (fp8), 1 DoubleRow matmul accumulates X2
At the end the two slabs are copied PSUM->SBUF on the scalar and vector
engines in parallel and DMA'd out on separate queues.
A short burst of dummy matmuls during the DMA prologue keeps the PE's HAM
activity window hot so the loop starts at the full 2.4GHz clock.
The four X1 accumulators share two PSUM banks: the jc0/jc2 matmul starts
each bank's accumulation group (pending-zeroing the whole bank) and the
jc1/jc3 matmuls never start, relying on PE program order for first-write
zeroing — CoreSim's group checker verifies this ordering at build time.

Sharding: data-parallel over j across 8 cores (512 rows each). Every core
holds the full xT bf16; the scalar loss is reduced on host (the
"all-reduce" of a scalar).
"""

import numpy as np

N, D, KK, C = 4096, 128, 4, 64
NCORES = 8
JSH = N // NCORES          # 512 j rows per core
JCH = 4                    # j-chunks of 128 per core
LT = N // 128              # 32 l-tiles
LTP = LT // 2              # 16 l-tile pairs
QW = 192                   # x1 width: P(64) | W0(64) | U(64)
ACCW = 256                 # per-jc accumulator stride (half a PSUM bank)
NSEL = 3                   # selections per row from X1
AUXW = 68                  # aux cols: 64 one-hot + 3 ktab + 1 pad
EPS = 1e-6
OMEGA = 0.1
KSC = float(np.float32(np.exp(-3.5)))        # Ln input scale (exactly f32)
SHIFT = float(-np.log(np.float64(KSC)))      # effective shift s = -ln(KSC)

_CACHE = {}


def _build_nc():
    import concourse.bass as bass
    import concourse.bacc as bacc
    import concourse.mybir as mybir
    import concourse.tile as tile
    from contextlib import ExitStack

    f32 = mybir.dt.float32
    bf16 = mybir.dt.bfloat16
    fp8 = mybir.dt.float8e4
    DR = mybir.MatmulPerfMode.DoubleRow
    Ln = mybir.ActivationFunctionType.Ln
    mult = mybir.AluOpType.mult
    add = mybir.AluOpType.add
    AxX = mybir.AxisListType.X

    nc = bacc.Bacc("TRN2", target_bir_lowering=False, debug=False)
    xt = nc.dram_tensor("xt", [D, N], bf16, kind="ExternalInput")
    # q pairs: row p holds [q(l0) | q(l1)] per l-tile pair, fp8
    qp = nc.dram_tensor("qp", [128, LT * QW], fp8, kind="ExternalInput")
    # V pairs: row p holds [v(l0) | v(l1)] per l-tile pair, fp8
    vp = nc.dram_tensor("vp", [128, LT * C], fp8, kind="ExternalInput")
    # raw X1 / X2 slabs go to host; gathers + bilinear term are O(N) there
    x1out = nc.dram_tensor("x1out", [128, JCH * QW], bf16, kind="ExternalOutput")
    xvout = nc.dram_tensor("xvout", [C, JSH], bf16, kind="ExternalOutput")

    with tile.TileContext(nc) as tc, ExitStack() as ctx:
        cpool = ctx.enter_context(tc.tile_pool(name="const", bufs=1))
        work = ctx.enter_context(tc.tile_pool(name="work", bufs=5))
        small = ctx.enter_context(tc.tile_pool(name="small", bufs=2))
        psim = ctx.enter_context(tc.tile_pool(name="psim", bufs=2, space="PSUM"))
        pacc = ctx.enter_context(tc.tile_pool(name="pacc", bufs=1, space="PSUM"))

        # ---- constants in SBUF; first-needed bytes land first ----
        xt_sb = cpool.tile([D, N], bf16, tag="xt")
        qp_sb = cpool.tile([128, LT * QW], fp8, tag="qp")
        vp_sb = cpool.tile([128, LT * C], fp8, tag="vp")
        nc.sync.dma_start(xt_sb[:, 0:512], xt[:, 0:512])
        nc.sync.dma_start(qp_sb[:, 0 : 4 * QW], qp[:, 0 : 4 * QW])
        nc.sync.dma_start(vp_sb[:, 0 : 4 * C], vp[:, 0 : 4 * C])
        nc.sync.dma_start(xt_sb[:, 512:1024], xt[:, 512:1024])
        nc.sync.dma_start(xt_sb[:, 1024:2048], xt[:, 1024:2048])
        nc.sync.dma_start(qp_sb[:, 4 * QW : 12 * QW], qp[:, 4 * QW : 12 * QW])
        nc.sync.dma_start(vp_sb[:, 4 * C :], vp[:, 4 * C :])
        nc.sync.dma_start(xt_sb[:, 2048:3072], xt[:, 2048:3072])
        nc.sync.dma_start(qp_sb[:, 12 * QW : 22 * QW], qp[:, 12 * QW : 22 * QW])
        nc.sync.dma_start(xt_sb[:, 3072:4096], xt[:, 3072:4096])
        nc.sync.dma_start(qp_sb[:, 22 * QW :], qp[:, 22 * QW :])
        qp3 = qp_sb[:].rearrange("p (t s c) -> p t s c", t=LTP, s=2)
        vp3 = vp_sb[:].rearrange("p (t s c) -> p t s c", t=LTP, s=2)

        # ---- accumulators: 4 X1 chunks packed two-per-bank (jc0/jc2 start
        # the bank's group, jc1/jc3 ride along), X2 in its own bank ----
        acc = pacc.tile([128, JCH * ACCW], f32, tag="acc")
        xv = pacc.tile([C, JSH], f32, tag="xv")

        # ---- PE warm-up in the spare PSUM bank: keep the HAM activity
        # window hot through the DMA prologue so the loop starts at full
        # clock instead of ramping through its first ~5us ----
        warm_sb = cpool.tile([128, 256], bf16, tag="warm")
        nc.gpsimd.memset(warm_sb[:], 0.0)
        warmp = pacc.tile([128, 256], f32, tag="warmp")
        for _ in range(10):
            nc.tensor.matmul(
                warmp[:], warm_sb[:, 0:128], warm_sb[:], start=True, stop=True
            )

        for ltp in range(LTP):
            st = ltp == 0
            sp = ltp == LTP - 1
            simp = psim.tile([128, 2 * JSH], f32, tag="simp", name=f"simp{ltp}")
            for s in range(2):
                lt = 2 * ltp + s
                nc.tensor.matmul(
                    simp[:, s * JSH : (s + 1) * JSH],
                    xt_sb[:, bass.ts(lt, 128)],
                    xt_sb[:, 0:JSH],
                    start=True,
                    stop=True,
                )
            lsp = work.tile([128, 2 * JSH], fp8, tag="lsp", name=f"lsp{ltp}")
            nc.scalar.activation(lsp[:], simp[:], Ln, scale=KSC)
            lsp3 = lsp[:].rearrange("p (s j) -> p s j", s=2)
            for jc in range(JCH):
                nc.tensor.matmul(
                    acc[:, jc * ACCW : jc * ACCW + QW],
                    lsp3[:, :, bass.ts(jc, 128)],
                    qp3[:, ltp],
                    start=st and jc % 2 == 0,
                    stop=sp and jc % 2 == 1,
                    perf_mode=DR,
                )
            ls2p = work.tile([128, 2 * JSH], fp8, tag="ls2p", name=f"ls2p{ltp}")
            nc.vector.tensor_mul(ls2p[:], lsp[:], lsp[:])
            ls2p3 = ls2p[:].rearrange("p (s j) -> p s j", s=2)
            nc.tensor.matmul(
                xv[:], vp3[:, ltp], ls2p3, start=st, stop=sp, perf_mode=DR
            )

        # ---- raw slabs out: copies run on ACT and DVE in parallel ----
        ob1 = small.tile([128, JCH * QW], bf16, tag="ob1")
        nc.scalar.copy(
            ob1[:].rearrange("p (j c) -> p j c", j=JCH),
            acc[:].rearrange("p (j c) -> p j c", j=JCH)[:, :, 0:QW],
        )
        nc.sync.dma_start(x1out[:], ob1[:])
        ob2 = small.tile([C, JSH], bf16, tag="ob2")
        nc.vector.tensor_copy(ob2[:], xv[:])
        nc.sync.dma_start(xvout[:], ob2[:])
    nc.compile()
    return nc


def _host_prep(inputs, labels):
    import ml_dtypes

    bf16 = ml_dtypes.bfloat16
    fp8 = ml_dtypes.float8_e4m3
    x = np.ascontiguousarray(np.asarray(inputs, dtype=np.float32))
    lab = np.asarray(labels)
    t = lab[:, 0]

    m = np.arange(KK)
    lp = np.log(np.float64(OMEGA + EPS)) - np.log(
        np.float64(OMEGA) ** (KK - m + 1) + np.float64(EPS)
    )

    gr = np.arange(C)
    eq = lab[None, :, :] == gr[:, None, None]          # [C, N, KK]
    nm = np.stack(
        [
            ~eq[:, :, 3],
            eq[:, :, 3] & ~eq[:, :, 2],
            eq[:, :, 2] & ~eq[:, :, 1],
            eq[:, :, 1] & ~eq[:, :, 0],
        ]
    ).astype(np.float64)                                # [KK, C, N]
    w0 = nm.sum(0)                                      # [C, N]
    w1 = np.einsum("m,mcl->cl", lp, nm)
    w2 = np.einsum("m,mcl->cl", lp * lp, nm)
    ph = (t[:, None] == gr[None, :]).astype(np.float64)  # [N, C] one-hot t_l

    h0 = w0.sum(1)
    h1 = w1.sum(1)
    h2 = w2.sum(1)
    cnt = ph.sum(0)
    s = np.float64(SHIFT)

    # diag in logsim-domain from the bf16-rounded x the device will see
    xb = x.astype(bf16).astype(np.float64)
    diag = np.log((xb * xb).sum(1) + EPS)                # [N]

    a0 = h0[t]
    cc = cnt[t] - 1.0
    u1 = s * cnt[t] - diag
    u2 = s * s * cnt[t] - diag * diag
    u3 = s * h0[t] + 0.1 * h1[t]
    u4 = s * s * h0[t] + 0.2 * s * h1[t] + 0.01 * h2[t]
    k0 = (a0 * u2 - 2.0 * u1 * u3 + cc * u4).sum()       # host constant
    k2 = 2.0 * diag - 2.0 * s

    # fold t_j-only linear terms into U (for X1) and V (for X2), scaled
    # into fp8e4 range (TRN max-normal 240)
    U = -0.2 * h1[None, :] * ph + 0.2 * (cnt[None, :] - 1.0) * w1.T
    V = h0[None, :] * ph + (cnt[None, :] - 1.0) * w0.T
    su = 2.0 ** np.floor(np.log2(200.0 / np.abs(U).max()))
    sv = 2.0 ** np.floor(np.log2(200.0 / np.abs(V).max()))

    qm = np.concatenate([ph, w0.T, U * su], axis=1).astype(fp8)  # [N, 192]
    vm = (V * sv).astype(fp8)                                    # [N, 64]

    xt = np.ascontiguousarray(x.T.astype(bf16))          # [D, N] bf16
    in_maps = []
    for cid in range(NCORES):
        sl = slice(cid * JSH, (cid + 1) * JSH)
        # rotate the l axis so this core's own j-shard sits at columns
        # 0:JSH — the kernel always matmuls against xt[:, 0:512]; the l
        # reduction (over all 4096) is rotation-invariant as long as q/v
        # rows rotate identically.
        xtc = np.ascontiguousarray(np.roll(xt, -cid * JSH, axis=1))
        # pair images [128, ltp, s, width]
        qpc = np.ascontiguousarray(
            np.roll(qm, -cid * JSH, axis=0)
            .reshape(LTP, 2, 128, QW).transpose(2, 0, 1, 3)
            .reshape(128, LT * QW)
        )
        vpc = np.ascontiguousarray(
            np.roll(vm, -cid * JSH, axis=0)
            .reshape(LTP, 2, 128, C).transpose(2, 0, 1, 3)
            .reshape(128, LT * C)
        )
        in_maps.append({"xt": xtc, "qp": qpc, "vp": vpc})
    host = {"k0": float(k0), "k2": k2, "t": t, "ph": ph,
            "su": float(su), "sv": float(sv)}
    return in_maps, host


def _run(inputs, labels, trace=False, tmpdir=None):
    from concourse.bass_utils import run_bass_kernel_spmd

    if "nc" not in _CACHE:
        _CACHE["nc"] = _build_nc()
    in_maps, host = _host_prep(inputs, labels)
    res = run_bass_kernel_spmd(
        _CACHE["nc"], in_maps, core_ids=list(range(NCORES)),
        trace=trace, tmpdir=tmpdir,
    )
    k2, t, ph = host["k2"], host["t"], host["ph"]
    su, sv = host["su"], host["sv"]
    loss = np.float64(host["k0"])
    jj = np.arange(JSH)
    for cid, r in enumerate(res.results):
        sl = slice(cid * JSH, (cid + 1) * JSH)
        # j-major slab: row p, block jc holds X1[j = jc*128 + p, :]
        a = (r["x1out"].astype(np.float64)
             .reshape(128, JCH, QW).transpose(1, 0, 2).reshape(JSH, QW))
        tj = t[sl]
        s1p = a[jj, tj]
        s1w0 = a[jj, 64 + tj]
        selu = a[jj, 128 + tj] / su
        loss += (-2.0 * s1p * s1w0 + k2[sl] * s1w0 + selu).sum()
        loss += (r["xvout"].astype(np.float64) * ph[sl].T).sum() / sv
    return np.array(loss, dtype=np.float32), res


def kernel(inputs, labels):
    out, _ = _run(inputs, labels, trace=False)
    return out
